# revision 47
# baseline (speedup 1.0000x reference)
"""Trainium2 Bass kernel for nn_NeuralAttention (MLP-scored attention).

Math (per head h, batch 1, n=512, dh=64, P=32):
  qkv = x @ Wqkv^T, split 'b n (d k h) -> k b h n d'
  qp = q@Wq^T+bq ; kp = k@Wk^T+bk
  a  = qp@W1q^T  ; c = kp@W1k^T          (W1 = [W1q | W1k])
  h1 = relu(a_i + c_j + b1)              # [n, n, 32]
  h2 = relu(h1 @ W2^T + b2)              # [n, n, 16]
  s  = h2 @ W3^T (+ b3, drops in softmax)
  attn = softmax(causal(s)) ; out = attn @ v ; y = out @ Wout^T

Key algebra: Aq = W1q@Wq, Ak = W1k@Wk, s1const = W1q bq + W1k bk + b1.

Sharding: 16 heads over 8 cores (2 heads/core), Wout row-parallel; host
sums the 16 partial [1024, 512] bf16 outputs (2 per core) in float64.

fp8 usage (validated in numpy vs the fp32 reference, and on device:
rel err 3.6e-3 vs the 2e-2 gate):
  - q/k projection: x*SX and Wqkv*SW in fp8e4, DoubleRow matmuls
    (K=256/instr at 0.5 cyc/col = 4x bf16); F=SX*SW descale is folded
    into Aq/Ak host-side.
  - h1 carries scale S1 (folded into aqrep/akT/s1c) so fp8 h1 lands in
    e4m3's normal range; W2 is scaled by S4/S1 so the stage-2 psum is
    S4*(h1@W2^T) regardless of the h1 path taken.
  - stage-2 per j-pair takes one of two routes, chosen round-robin so
    all four engines stay busy: bf16 h1 on DVE (4x tensor_scalar mode)
    + 2 plain matmuls, or fp8 h1 on GPSIMD + ONE DoubleRow matmul with
    block-diagonal fp8 W2 (w2d), 4x fewer PE cycles.
  - stage-2.5 emits h2 = relu(psum + S4 b2) in fp8e4 (Act mostly, some
    DVE); dtype is free on those engines.
  - stage-3: DoubleRow fp8 matmuls; scatter weights w3f = W3*(SC/S4)
    are full-height [128, 2, 128] per (ab, parity) because DR codegen
    requires output base partition 0; each matmul accumulates two
    j-pairs (16 j) of scores.
  - exp applies scale 1/SC; the causal mask (-1e30, bf16
    identity-matmul init of the score psum) stays enormous after the
    1/SC scaling so masked lanes exp to exactly 0.

On-device layout ("j on partitions"): scores^T[j, i] in j-tiles of 128;
pair m (8 j) evaluates i >= 8*(m&~1) (causal, quantized to DR pairs).
PSUM discipline: every psum tile fits one 2KB bank (multi-bank views
break), hence per-pair ps2 tiles for L>256 and merged pairs below.
Softmax denominator comes from a ones-column in the attn@v matmul;
normalization multiplies num (still in PSUM) by the broadcast
reciprocal. P4 (out proj) is emitted as deferred units drained into
the other head's scoring to fill PE bubbles; outputs are per-head bf16
partials DMA'd in 4 large transfers. Input DMAs are consolidated into
~10 loads (HWDGE issue is 625ns each, serialized).

Modeled (TimelineSim, calibrated cost model) per-core time: 66.8 us
(baseline 77.7); engine busy ~ Act 48 / DVE 45 / Pool 39 / PE 34 us —
the elementwise relu stages (8.7M + 4.3M elems) across DVE/Act/GPSIMD
are the wall, with PE pulled well below it by the fp8 DoubleRow paths.
"""

import sys

sys.path.insert(0, "/opt/trn_rl_repo")

from contextlib import ExitStack

import ml_dtypes
import numpy as np

import concourse.bass as bass
import concourse.tile as tile
from concourse import bacc, mybir
from concourse.bass_utils import run_bass_kernel_spmd

F32 = mybir.dt.float32
F16 = mybir.dt.float16
BF16 = mybir.dt.bfloat16
F8 = mybir.dt.float8e4
AF = mybir.ActivationFunctionType
ALU = mybir.AluOpType
DR = mybir.MatmulPerfMode.DoubleRow

B, N, DIM = 1, 512, 1024
HEADS, DH = 16, 64
P, P2 = 32, 16
N_CORES = 8
HPC = HEADS // N_CORES  # heads per core = 2
NT = N // 128            # j tiles = 4
KT = DIM // 128          # contraction tiles for projections = 8

SX, SW = 16.0, 512.0     # fp8 scales: x, Wqkv (qk slice)
S1 = 64.0                # h1 scale (folded into aqrep/akT/s1c, out of w2b)
S4 = 512.0               # h2 scale (folded into w2b/b2r)
SC = 2048.0              # score scale; exp uses 1/SC

# scheduling tunables
TUNE = dict(
    s2_bufs=4,       # stage-2 psum tiles
    h1_bufs=16,      # stage-1 sbuf tiles
    h2_bufs=13,       # h2 sbuf tiles
    ex_bufs=5,       # exp sbuf tiles
    f8_pair_mod=3,   # every Nth j-pair: fp8 h1 on Pool + DoubleRow stage-2
    s1_act_mod=0,    # every Nth bf16 stage-1 op -> Act (0 = none)
    s25_dve_mod=3,   # every Nth stage-2.5 op -> DVE (0 = none)
)


# ---------------------------------------------------------------- program ---

def build_program(repeat: int = 1):
    nc = bacc.Bacc("TRN2", target_bir_lowering=False, debug=False,
                   num_devices=N_CORES)

    d = {}
    def din(name, shape, dt):
        d[name] = nc.dram_tensor(name, shape, dt, kind="ExternalInput").ap()
        return d[name]

    x8_d = din("x8", [128, KT * N], F8)          # x*SX transposed, fp8
    wqk8_d = din("wqk8", [128, KT * 4 * DH], F8)  # [q_h0 q_h1 k_h0 k_h1]*SW
    xT_d = din("xT", [DIM, N], BF16)             # x transposed (v proj)
    wvT_d = din("wvT", [DIM, HPC * DH], BF16)    # v rhs (both heads)
    cstB_d = din("cstB", [128, 864], BF16)       # aqrep|akT|w2b|iden|tri
    cstF_d = din("cstF", [128, 2], F32)          # s1c | b2r
    w3f_d = din("w3f", [128, 8 * 256], F8)       # (ab,pi) scatter weights
    w2d_d = din("w2d", [128, 256], F8)           # DR stage-2 blockdiag
    wo2_d = din("wo2", [128, DIM], BF16)         # packed Wout slice lhsT

    outT_d = [nc.dram_tensor(f"outT{h}", [DIM, N], BF16,
                              kind="ExternalOutput").ap() for h in range(HPC)]

    with tile.TileContext(nc) as tc, ExitStack() as ctx:
        cst = ctx.enter_context(tc.tile_pool(name="cst", bufs=1))

        # --- consolidated input DMAs (order matters: qk path first) ---
        x8 = cst.tile([128, KT * N], F8, tag="x8")
        wqk8 = cst.tile([128, KT * 4 * DH], F8, tag="wqk8")
        nc.sync.dma_start(wqk8[:], wqk8_d[:])
        nc.sync.dma_start(x8[:, 0:KT * N // 2], x8_d[:, 0:KT * N // 2])
        nc.sync.dma_start(x8[:, KT * N // 2:], x8_d[:, KT * N // 2:])
        cstF = cst.tile([128, 2], F32, tag="cstF")
        nc.sync.dma_start(cstF[:], cstF_d[:])
        cstB = cst.tile([128, 864], BF16, tag="cstB")
        nc.sync.dma_start(cstB[:], cstB_d[:])
        w3f_big = cst.tile([128, 8 * 256], F8, tag="w3f")
        nc.sync.dma_start(w3f_big[:], w3f_d[:])
        w2d = cst.tile([128, 256], F8, tag="w2d")
        nc.sync.dma_start(w2d[:], w2d_d[:])
        x_big = cst.tile([128, KT * N], BF16, tag="xT16")
        nc.sync.dma_start(x_big[:], xT_d.rearrange("(a p) n -> p a n", p=128))
        wv_big = cst.tile([128, KT * HPC * DH], BF16, tag="wv")
        nc.sync.dma_start(wv_big[:], wvT_d.rearrange("(a p) m -> p a m", p=128))
        woutT = []
        for h in range(HPC):
            t = cst.tile([DH, DIM], BF16, tag=f"woutT_{h}")
            nc.sync.dma_start(t[:], wo2_d[DH * h:DH * (h + 1), :])
            woutT.append(t)

        aqrep = cstB[:, 0:128]
        akT = cstB[:, 128:160]
        w2b = cstB[:, 160:224]
        iden = cstB[:, 224:352]
        tri = cstB[:, 352:864]
        s1c = cstF[:, 0:1]
        b2r = cstF[:, 1:2]
        w3f = [w3f_big[:, k * 256:(k + 1) * 256] for k in range(8)]
        xT16 = [x_big[:, kk * N:(kk + 1) * N] for kk in range(KT)]
        wv = [wv_big[:, kk * HPC * DH:(kk + 1) * HPC * DH] for kk in range(KT)]

        # exp table warm-up
        warm = cst.tile([1, 4], F32, tag="warm")
        nc.vector.memset(warm[:], 0.0)
        nc.scalar.activation(warm[:], warm[:], AF.Exp)

        for rep in range(repeat):
            _body(nc, tc, ctx, rep, x8, wqk8, xT16, wv, aqrep, akT, s1c,
                  w2b, w2d, b2r, w3f, tri, iden, woutT, outT_d)

    nc.compile()
    return nc


def _body(nc, tc, ctx, rep, x8, wqk8, xT16, wv, aqrep, akT, s1c, w2b, w2d,
          b2r, w3f, tri, iden, woutT, outT_d):
    r = f"r{rep}"
    cst2 = ctx.enter_context(tc.tile_pool(name=f"cst2_{r}", bufs=1))

    # ------ P1: q/k projections (fp8 DoubleRow) -> qk16 [128, N] bf16 ------
    qk16 = []  # [q(2 heads), k(2 heads)]
    with tc.tile_pool(name=f"qkps_{r}", bufs=2, space="PSUM") as qkps:
        for m in range(2):
            ps = qkps.tile([128, N], F32, tag="qk")
            for kp in range(KT // 2):
                lhs = wqk8[:, kp * 512:(kp + 1) * 512] \
                    .rearrange("p (two c) -> p two c", two=2) \
                    [:, :, m * 128:(m + 1) * 128]
                rhs = x8[:, kp * 2 * N:(kp + 1) * 2 * N] \
                    .rearrange("p (two n) -> p two n", two=2)
                nc.tensor.matmul(ps[:, :], lhs, rhs,
                                 start=(kp == 0), stop=(kp == KT // 2 - 1),
                                 perf_mode=DR)
            sb = cst2.tile([128, N], BF16, tag=f"qk16_{m}")
            nc.vector.tensor_copy(sb[:], ps[:])
            qk16.append(sb)

    # -------- P3: score MLP + softmax + attn@v, heads interleaved ----------
    out_h = []  # [64, N] bf16 normalized attention output per head
    with tc.tile_pool(name=f"s2_{r}", bufs=TUNE["s2_bufs"], space="PSUM") as s2ps, \
         tc.tile_pool(name=f"sc_{r}", bufs=1, space="PSUM") as scps, \
         tc.tile_pool(name=f"op_{r}", bufs=1, space="PSUM") as ops, \
         tc.tile_pool(name=f"wk_{r}", bufs=TUNE["h1_bufs"]) as wk, \
         tc.tile_pool(name=f"h2_{r}", bufs=TUNE["h2_bufs"]) as h2p, \
         tc.tile_pool(name=f"ex_{r}", bufs=TUNE["ex_bufs"]) as exp_pool:

        a4s, cbs, op_pss = [], [], []
        for h in range(HPC):
            # a4 = 4x-replicated a^T (+ s1const via scalar add) [128, N] bf16
            a_ps = scps.tile([128, N], F32, tag=f"sc{h}")
            nc.tensor.matmul(a_ps[:, :], aqrep[64 * h:64 * (h + 1), :],
                             qk16[0][64 * h:64 * (h + 1), :],
                             start=True, stop=True, tile_position=(64 * h, 0))
            a4 = cst2.tile([128, N], BF16, tag=f"a4_{h}")
            nc.vector.tensor_scalar(a4[:], a_ps[:], s1c[:], None, ALU.add)
            a4s.append(a4)

            # cbias[32u+p, g] = (Ak k^T)[p, 4g+u]  [128, 128] f32
            c_ps = scps.tile([128, 128], F32, tag=f"sc{h}")
            k_re = qk16[1][64 * h:64 * (h + 1), :].rearrange(
                "d (g u) -> d u g", u=4)
            for u in range(4):
                nc.tensor.matmul(c_ps[32 * u:32 * (u + 1), :],
                                 akT[64 * h:64 * (h + 1), :],
                                 k_re[:, u, :], start=True, stop=True,
                                 tile_position=(64 * h, 32 * u))
            cb = cst2.tile([128, 128], F32, tag=f"cb_{h}")
            nc.vector.tensor_copy(cb[:], c_ps[:])
            cbs.append(cb)

            # out' accumulator [65, N] psum (num rows 0..64, den row 64)
            op_ps = ops.tile([65, N], F32, tag=f"op{h}")
            op_pss.append(op_ps)

        # ---- v projection -> v' [128, 130] bf16 per j-tile (emitted
        # lazily inside the scoring loop to keep the early PE stream free) --
        vp = cst2.tile([128, NT * 130], BF16, tag="vp")

        def emit_vproj(t):
            ps_v = s2ps.tile([128, HPC * DH], F32, tag="s2")
            for kk in range(KT):
                nc.tensor.matmul(ps_v[:, :],
                                 xT16[kk][:, t * 128:(t + 1) * 128],
                                 wv[kk][:, :],
                                 start=(kk == 0), stop=(kk == KT - 1))
            for h in range(HPC):
                o0 = t * 130 + h * 65
                nc.scalar.copy(vp[:, o0:o0 + DH],
                               ps_v[:, h * DH:(h + 1) * DH])
                nc.vector.memset(vp[:, o0 + DH:o0 + 65], 1.0)

        s1_n = [0]  # stage-1 round-robin counter
        pair_n = [0]
        s25_n = [0]
        deferred = []  # P4 work units for finished heads, drained during
                       # the other head's scoring to fill PE bubbles
        # greedy engine load balancing: estimated busy-ns per engine.
        # Act/Pool start idle until the qk chain completes - handicap them
        # so the balancer sees wall-clock finishing times, not raw load.
        est = {"dve": 0.0, "act": 0.0, "pool": 0.0}

        def bal_copy(dst, src, cols):
            act_c = 0.833 * cols + 185
            dve_c = 1.04 * cols + 125
            if est["dve"] + dve_c < est["act"] + act_c:
                est["dve"] += dve_c
                nc.vector.tensor_copy(dst, src)
            else:
                est["act"] += act_c
                nc.scalar.copy(dst, src)

        def emit_s1(h1, a4, i0ofs, cb, g, eng):
            if eng == "act":
                nc.scalar.activation(h1, a4[:, i0ofs:N], AF.Relu,
                                     bias=cb[:, g:g + 1], scale=1.0)
            else:
                e = nc.gpsimd if eng == "pool" else nc.vector
                e.tensor_scalar(h1, a4[:, i0ofs:N], cb[:, g:g + 1], 0.0,
                                ALU.add, ALU.max)

        def emit_s25(h2ap, psap, cols):
            s25_n[0] += 1
            act_c = 0.833 * cols + 185
            dve_c = 1.04 * cols + 125
            if est["dve"] + dve_c < est["act"] + act_c:
                est["dve"] += dve_c
                nc.vector.tensor_scalar(h2ap, psap, b2r[:], 0.0,
                                        ALU.add, ALU.max)
            else:
                est["act"] += act_c
                nc.scalar.activation(h2ap, psap, AF.Relu, bias=b2r[:],
                                     scale=1.0)

        for h in range(HPC):
            for t in range(NT):
                a4, cb, op_ps = a4s[h], cbs[h], op_pss[h]
                L = N - t * 128
                i0 = t * 128
                sc_ps = scps.tile([128, L], F32, tag=f"sc{h}")
                # causal mask init (-1e30 above diagonal); stage-3 accumulates.
                nc.tensor.matmul(sc_ps[:, :], iden[:, :], tri[:, 0:L],
                                 start=True, stop=False,
                                 skip_group_check=True)
                # nm = pairs per stage-2 psum tile (2 = DR pairing unit);
                # psum tile must stay within one 2KB bank (<=512 f32 cols).
                nm = 2 if L <= 256 else 1
                s3q = []   # deferred stage-3 emissions (1 m0-double late,
                           # so PE's s2 stream never stalls on Act's s25)
                for m0 in range(0, 16, 2):
                    if deferred and m0 % 4 == 2:
                        deferred.pop(0)()
                    ofs = 8 * m0
                    Lm = L - ofs
                    h2t = h2p.tile([128, 2 * Lm], F8, tag="h2",
                                   name=f"h2t_{h}_{t}_{m0}")

                    def emit_s2(ps2ap, m, Lm, ofs):
                        # stage-1 + stage-2 for pair m into ps2ap [128, Lm]
                        pair_n[0] += 1
                        pool_c = 2 * (1.39 * Lm + 95)
                        dve_c = 2 * (0.26 * Lm + 60)
                        use_pool = (est["pool"] + pool_c
                                    < est["dve"] + dve_c)
                        if use_pool:
                            # fp8 h1 on Pool + one DoubleRow stage-2 matmul
                            est["pool"] += pool_c
                            h1p = wk.tile([128, 2 * Lm], F8, tag="h1f",
                                          name=f"h1f_{pair_n[0]}")
                            for v in range(2):
                                g = 32 * t + 2 * m + v
                                emit_s1(h1p[:, v * Lm:(v + 1) * Lm], a4,
                                        i0 + ofs, cb, g, "pool")
                            nc.tensor.matmul(
                                ps2ap,
                                w2d[:].rearrange("p (two c) -> p two c",
                                                 two=2),
                                h1p[:].rearrange("p (two n) -> p two n",
                                                 two=2),
                                start=True, stop=True, perf_mode=DR)
                        else:
                            est["dve"] += dve_c
                            for v in range(2):
                                g = 32 * t + 2 * m + v
                                h1 = wk.tile([128, Lm], BF16, tag="h1",
                                             name=f"h1_{pair_n[0]}_{v}")
                                emit_s1(h1[:], a4, i0 + ofs, cb, g, "dve")
                                nc.tensor.matmul(
                                    ps2ap[64 * v:64 * (v + 1), :],
                                    w2b[:, :], h1[:], start=True, stop=True)

                    if nm == 1:
                        for half in range(2):       # pair m0+half
                            ps2 = s2ps.tile([128, Lm], F32, tag="s2")
                            emit_s2(ps2[:, :], m0 + half, Lm, ofs)
                            emit_s25(h2t[:, half * Lm:(half + 1) * Lm],
                                     ps2[:], Lm)
                    else:
                        ps2 = s2ps.tile([128, 2 * Lm], F32, tag="s2")
                        for dm in range(2):
                            emit_s2(ps2[:, dm * Lm:(dm + 1) * Lm],
                                    m0 + dm, Lm, ofs)
                        emit_s25(h2t[:], ps2[:], 2 * Lm)
                    # stage-3: one DoubleRow matmul for pair (m0, m0+1)
                    ab, pi = m0 // 4, (m0 // 2) % 2
                    nc.tensor.matmul(
                        sc_ps[:, ofs:L],
                        w3f[2 * ab + pi].rearrange(
                            "p (two c) -> p two c", two=2),
                        h2t[:].rearrange("p (two n) -> p two n", two=2),
                        start=False, stop=(m0 + 2 >= 16),
                        skip_group_check=True, perf_mode=DR)
                if h == 0:
                    emit_vproj(t)
                ex = exp_pool.tile([128, L], BF16, tag="ex")
                est["act"] += 0.833 * L + 185
                nc.scalar.activation(ex[:], sc_ps[:], AF.Exp, scale=1.0 / SC)
                nc.tensor.matmul(op_ps[:, i0:N],
                                 vp[:, t * 130 + h * 65: t * 130 + h * 65 + 65],
                                 ex[:], start=(t == 0), stop=(t == NT - 1),
                                 skip_group_check=True)
                if t == NT - 1:
                    # normalize this head: out = num * (1/den)
                    rsb = cst2.tile([128, N], F32, tag=f"rec_{h}")
                    nc.vector.reciprocal(rsb[64:65, :], op_ps[64:65, :])
                    ones = cst2.tile([128, DH], F32, tag=f"ones_{h}")
                    nc.vector.memset(ones[64:65, :], 1.0)
                    rb_ps = scps.tile([DH, N], F32, tag=f"sc{h}")
                    nc.tensor.matmul(rb_ps[:, :], ones[64:65, :],
                                     rsb[64:65, :], start=True, stop=True)
                    rb16 = cst2.tile([DH, N], BF16, tag=f"rb16_{h}")
                    nc.scalar.copy(rb16[:], rb_ps[:])
                    o = cst2.tile([DH, N], BF16, tag=f"out_{h}")
                    nc.vector.tensor_mul(o[:], op_ps[0:DH, :], rb16[:])
                    out_h.append(o)

                    def make_p4(h, o):
                        state = {}
                        def p4_unit(ot):
                            ps = s2ps.tile([128, N], F32, tag="s2")
                            nc.tensor.matmul(
                                ps[:, :],
                                woutT[h][:, ot * 128:(ot + 1) * 128],
                                o[:, :], start=True, stop=True)
                            if ot % 2 == 0:
                                state["ob"] = wk.tile(
                                    [128, 2 * N], BF16, tag=f"ob{h}",
                                    name=f"obt_{h}_{ot}")
                                nc.vector.tensor_copy(
                                    state["ob"][:, 0:N], ps[:])
                            else:
                                ob = state["ob"]
                                nc.scalar.copy(ob[:, N:2 * N], ps[:])
                                nc.sync.dma_start(
                                    outT_d[h].rearrange(
                                        "(c a p) n -> p c a n", p=128, c=4)
                                    [:, ot // 2],
                                    ob[:].rearrange("p (a n) -> p a n", a=2))
                        return [lambda ot=ot: p4_unit(ot)
                                for ot in range(KT)]

                    deferred.extend(make_p4(h, o))

        while deferred:
            deferred.pop(0)()


# ---------------------------------------------------------------- host side -

def prep_inputs(x, Wqkv, Wout, Wq, bq, Wk, bk, W1, b1, W2, b2, W3, b3):
    """Build the per-core input maps (all numpy)."""
    x = np.asarray(x, np.float32).reshape(N, DIM)
    Wqkv = np.asarray(Wqkv, np.float32)
    Wout = np.asarray(Wout, np.float32)
    Wq, bq = np.asarray(Wq, np.float32), np.asarray(bq, np.float32)
    Wk, bk = np.asarray(Wk, np.float32), np.asarray(bk, np.float32)
    W1, b1 = np.asarray(W1, np.float32), np.asarray(b1, np.float32)
    W2, b2 = np.asarray(W2, np.float32), np.asarray(b2, np.float32)
    W3 = np.asarray(W3, np.float32)

    bf = lambda a: np.ascontiguousarray(a).astype(ml_dtypes.bfloat16)
    f8 = lambda a: np.ascontiguousarray(a).astype(ml_dtypes.float8_e4m3)
    f32 = lambda a: np.ascontiguousarray(a, np.float32)

    xT = x.T                                        # [DIM, N]
    # x8 fp8 layout [128, KT*N]: col kk*N+n, row p -> x[n, kk*128+p]*SX
    x8 = f8(xT.reshape(KT, 128, N).transpose(1, 0, 2).reshape(128, KT * N)
            * SX)

    F = SX * SW
    W1q, W1k = W1[:, :P], W1[:, P:]
    Aq = (W1q @ Wq) * (S1 / F)                      # descale fp8, scale S1
    Ak = (W1k @ Wk) * (S1 / F)
    s1const = (W1q @ bq + W1k @ bk + b1) * S1       # [32]

    aqrep = np.zeros((128, 128), np.float32)
    for u in range(4):
        aqrep[0:DH, 32 * u:32 * (u + 1)] = Aq.T
    aqrep[DH:128] = aqrep[0:DH]
    akT = np.concatenate([Ak.T, Ak.T], axis=0)      # [128, 32]

    w2b = np.zeros((128, 64), np.float32)     # blockdiag4((S4/S1)*W2^T)
    for u in range(4):
        w2b[32 * u:32 * (u + 1), 16 * u:16 * (u + 1)] = W2.T * (S4 / S1)
    # DR stage-2 weights: [128, 2, 128] fp8; i-half v targets rows 64v..
    w2drh = np.zeros((128, 2, 128), np.float32)
    for v in range(2):
        w2drh[:, v, 64 * v:64 * (v + 1)] = w2b[:, 0:64]

    ii = np.arange(128)
    tri = np.zeros((128, N), np.float32)        # [j, i]: 0 valid, -1e30 not
    tri[:, 0:128] = np.where(ii[None, :] >= ii[:, None], 0.0, -1e30)
    iden = np.eye(128, dtype=np.float32)

    cstB = np.concatenate(
        [aqrep, akT, w2b, iden, tri], axis=1)       # [128, 864]
    cstF = np.stack([np.tile(s1const, 4), np.tile(b2 * S4, 8)],
                    axis=1)                         # [128, 2] f32

    # full-height DR scatter weights: w3f[2*ab+pi][p, i, col] nonzero at
    # col = 32*ab + 8*bb + 4*v + u for bb = 2*pi + i, p = 64v+16u+q.
    w3v = W3[0] * (SC / S4)                         # [16]
    w3full = np.zeros((8, 128, 2, 128), np.float32)
    for ab in range(4):
        for pi in range(2):
            for i in range(2):
                bb = 2 * pi + i
                for v in range(2):
                    for u in range(4):
                        col = 32 * ab + 8 * bb + 4 * v + u
                        for q in range(P2):
                            w3full[2 * ab + pi, 64 * v + 16 * u + q, i,
                                   col] = w3v[q]

    # per-head channel index in Wqkv output: o = d*48 + k*16 + h
    dch = np.arange(DH)
    in_maps = []
    for c in range(N_CORES):
        h0, h1h = HPC * c, HPC * c + 1
        rows_q = [dch * 48 + 0 * HEADS + h for h in (h0, h1h)]
        rows_k = [dch * 48 + 1 * HEADS + h for h in (h0, h1h)]
        rows_v = [dch * 48 + 2 * HEADS + h for h in (h0, h1h)]
        wqkT = np.concatenate(
            [Wqkv[r] for r in rows_q + rows_k], axis=0).T     # [DIM, 256]
        # wqk8 fp8 layout [128, KT*4DH]
        wqk8 = f8(wqkT.reshape(KT, 128, 4 * DH).transpose(1, 0, 2)
                  .reshape(128, KT * 4 * DH) * SW)
        wvT = np.concatenate([Wqkv[r] for r in rows_v], axis=0).T  # [DIM,128]
        wo2 = np.concatenate(
            [Wout[:, DH * h:DH * (h + 1)].T for h in (h0, h1h)])  # [128,DIM]
        in_maps.append({
            "x8": x8,
            "wqk8": wqk8,
            "xT": bf(xT),
            "wvT": bf(wvT),
            "cstB": bf(cstB),
            "cstF": f32(cstF),
            "w3f": f8(w3full.transpose(1, 0, 2, 3).reshape(128, 8 * 256)),
            "w2d": f8(w2drh.reshape(128, 256)),
            "wo2": bf(wo2),
        })
    return in_maps


_PROGRAM_CACHE = {}


def _get_program(repeat=1):
    if repeat not in _PROGRAM_CACHE:
        _PROGRAM_CACHE[repeat] = build_program(repeat)
    return _PROGRAM_CACHE[repeat]


def run(in_maps, repeat=1):
    nc = _get_program(repeat)
    return run_bass_kernel_spmd(nc, in_maps, list(range(N_CORES)))


def kernel(**inputs) -> np.ndarray:
    in_maps = prep_inputs(**inputs)
    res = run(in_maps)
    acc = np.zeros((DIM, N), np.float64)
    for c in range(N_CORES):
        for h in range(HPC):
            acc += res.results[c][f"outT{h}"].astype(np.float64)
    return np.ascontiguousarray(acc.T.astype(np.float32)).reshape(B, N, DIM)


# revision 48
# speedup vs baseline: 1.0028x; 1.0028x over previous
"""Trainium2 Bass kernel for nn_NeuralAttention (MLP-scored attention).

Math (per head h, batch 1, n=512, dh=64, P=32):
  qkv = x @ Wqkv^T, split 'b n (d k h) -> k b h n d'
  qp = q@Wq^T+bq ; kp = k@Wk^T+bk
  a  = qp@W1q^T  ; c = kp@W1k^T          (W1 = [W1q | W1k])
  h1 = relu(a_i + c_j + b1)              # [n, n, 32]
  h2 = relu(h1 @ W2^T + b2)              # [n, n, 16]
  s  = h2 @ W3^T (+ b3, drops in softmax)
  attn = softmax(causal(s)) ; out = attn @ v ; y = out @ Wout^T

Key algebra: Aq = W1q@Wq, Ak = W1k@Wk, s1const = W1q bq + W1k bk + b1.

Sharding: 16 heads over 8 cores (2 heads/core), Wout row-parallel; host
sums the 16 partial [1024, 512] bf16 outputs (2 per core) in float64.

fp8 usage (validated in numpy vs the fp32 reference, and on device:
rel err 3.6e-3 vs the 2e-2 gate):
  - q/k projection: x*SX and Wqkv*SW in fp8e4, DoubleRow matmuls
    (K=256/instr at 0.5 cyc/col = 4x bf16); F=SX*SW descale is folded
    into Aq/Ak host-side.
  - h1 carries scale S1 (folded into aqrep/akT/s1c) so fp8 h1 lands in
    e4m3's normal range; W2 is scaled by S4/S1 so the stage-2 psum is
    S4*(h1@W2^T) regardless of the h1 path taken.
  - stage-2 per j-pair takes one of two routes, chosen round-robin so
    all four engines stay busy: bf16 h1 on DVE (4x tensor_scalar mode)
    + 2 plain matmuls, or fp8 h1 on GPSIMD + ONE DoubleRow matmul with
    block-diagonal fp8 W2 (w2d), 4x fewer PE cycles.
  - stage-2.5 emits h2 = relu(psum + S4 b2) in fp8e4 (Act mostly, some
    DVE); dtype is free on those engines.
  - stage-3: DoubleRow fp8 matmuls; scatter weights w3f = W3*(SC/S4)
    are full-height [128, 2, 128] per (ab, parity) because DR codegen
    requires output base partition 0; each matmul accumulates two
    j-pairs (16 j) of scores.
  - exp applies scale 1/SC; the causal mask (-1e30, bf16
    identity-matmul init of the score psum) stays enormous after the
    1/SC scaling so masked lanes exp to exactly 0.

On-device layout ("j on partitions"): scores^T[j, i] in j-tiles of 128;
pair m (8 j) evaluates i >= 8*(m&~1) (causal, quantized to DR pairs).
PSUM discipline: every psum tile fits one 2KB bank (multi-bank views
break), hence per-pair ps2 tiles for L>256 and merged pairs below.
Softmax denominator comes from a ones-column in the attn@v matmul;
normalization multiplies num (still in PSUM) by the broadcast
reciprocal. P4 (out proj) is emitted as deferred units drained into
the other head's scoring to fill PE bubbles; outputs are per-head bf16
partials DMA'd in 4 large transfers. Input DMAs are consolidated into
~10 loads (HWDGE issue is 625ns each, serialized).

Modeled (TimelineSim, calibrated cost model) per-core time: 66.8 us
(baseline 77.7); engine busy ~ Act 48 / DVE 45 / Pool 39 / PE 34 us —
the elementwise relu stages (8.7M + 4.3M elems) across DVE/Act/GPSIMD
are the wall, with PE pulled well below it by the fp8 DoubleRow paths.
"""

import sys

sys.path.insert(0, "/opt/trn_rl_repo")

from contextlib import ExitStack

import ml_dtypes
import numpy as np

import concourse.bass as bass
import concourse.tile as tile
from concourse import bacc, mybir
from concourse.bass_utils import run_bass_kernel_spmd

F32 = mybir.dt.float32
F16 = mybir.dt.float16
BF16 = mybir.dt.bfloat16
F8 = mybir.dt.float8e4
AF = mybir.ActivationFunctionType
ALU = mybir.AluOpType
DR = mybir.MatmulPerfMode.DoubleRow

B, N, DIM = 1, 512, 1024
HEADS, DH = 16, 64
P, P2 = 32, 16
N_CORES = 8
HPC = HEADS // N_CORES  # heads per core = 2
NT = N // 128            # j tiles = 4
KT = DIM // 128          # contraction tiles for projections = 8

SX, SW = 16.0, 512.0     # fp8 scales: x, Wqkv (qk slice)
S1 = 64.0                # h1 scale (folded into aqrep/akT/s1c, out of w2b)
S4 = 512.0               # h2 scale (folded into w2b/b2r)
SC = 2048.0              # score scale; exp uses 1/SC

# scheduling tunables
TUNE = dict(
    s2_bufs=5,       # stage-2 psum tiles
    h1_bufs=16,      # stage-1 sbuf tiles
    h2_bufs=13,       # h2 sbuf tiles
    ex_bufs=5,       # exp sbuf tiles
    f8_pair_mod=3,   # every Nth j-pair: fp8 h1 on Pool + DoubleRow stage-2
    s1_act_mod=0,    # every Nth bf16 stage-1 op -> Act (0 = none)
    s25_dve_mod=3,   # every Nth stage-2.5 op -> DVE (0 = none)
)


# ---------------------------------------------------------------- program ---

def build_program(repeat: int = 1):
    nc = bacc.Bacc("TRN2", target_bir_lowering=False, debug=False,
                   num_devices=N_CORES)

    d = {}
    def din(name, shape, dt):
        d[name] = nc.dram_tensor(name, shape, dt, kind="ExternalInput").ap()
        return d[name]

    x8_d = din("x8", [128, KT * N], F8)          # x*SX transposed, fp8
    wqk8_d = din("wqk8", [128, KT * 4 * DH], F8)  # [q_h0 q_h1 k_h0 k_h1]*SW
    xT_d = din("xT", [DIM, N], BF16)             # x transposed (v proj)
    wvT_d = din("wvT", [DIM, HPC * DH], BF16)    # v rhs (both heads)
    cstB_d = din("cstB", [128, 864], BF16)       # aqrep|akT|w2b|iden|tri
    cstF_d = din("cstF", [128, 2], F32)          # s1c | b2r
    w3f_d = din("w3f", [128, 8 * 256], F8)       # (ab,pi) scatter weights
    w2d_d = din("w2d", [128, 256], F8)           # DR stage-2 blockdiag
    wo2_d = din("wo2", [128, DIM], BF16)         # packed Wout slice lhsT

    outT_d = [nc.dram_tensor(f"outT{h}", [DIM, N], BF16,
                              kind="ExternalOutput").ap() for h in range(HPC)]

    with tile.TileContext(nc) as tc, ExitStack() as ctx:
        cst = ctx.enter_context(tc.tile_pool(name="cst", bufs=1))

        # --- consolidated input DMAs (order matters: qk path first) ---
        x8 = cst.tile([128, KT * N], F8, tag="x8")
        wqk8 = cst.tile([128, KT * 4 * DH], F8, tag="wqk8")
        nc.sync.dma_start(wqk8[:], wqk8_d[:])
        nc.sync.dma_start(x8[:, 0:KT * N // 2], x8_d[:, 0:KT * N // 2])
        nc.sync.dma_start(x8[:, KT * N // 2:], x8_d[:, KT * N // 2:])
        cstF = cst.tile([128, 2], F32, tag="cstF")
        nc.sync.dma_start(cstF[:], cstF_d[:])
        cstB = cst.tile([128, 864], BF16, tag="cstB")
        nc.sync.dma_start(cstB[:], cstB_d[:])
        w3f_big = cst.tile([128, 8 * 256], F8, tag="w3f")
        nc.sync.dma_start(w3f_big[:], w3f_d[:])
        w2d = cst.tile([128, 256], F8, tag="w2d")
        nc.sync.dma_start(w2d[:], w2d_d[:])
        x_big = cst.tile([128, KT * N], BF16, tag="xT16")
        nc.sync.dma_start(x_big[:], xT_d.rearrange("(a p) n -> p a n", p=128))
        wv_big = cst.tile([128, KT * HPC * DH], BF16, tag="wv")
        nc.sync.dma_start(wv_big[:], wvT_d.rearrange("(a p) m -> p a m", p=128))
        woutT = []
        for h in range(HPC):
            t = cst.tile([DH, DIM], BF16, tag=f"woutT_{h}")
            nc.sync.dma_start(t[:], wo2_d[DH * h:DH * (h + 1), :])
            woutT.append(t)

        aqrep = cstB[:, 0:128]
        akT = cstB[:, 128:160]
        w2b = cstB[:, 160:224]
        iden = cstB[:, 224:352]
        tri = cstB[:, 352:864]
        s1c = cstF[:, 0:1]
        b2r = cstF[:, 1:2]
        w3f = [w3f_big[:, k * 256:(k + 1) * 256] for k in range(8)]
        xT16 = [x_big[:, kk * N:(kk + 1) * N] for kk in range(KT)]
        wv = [wv_big[:, kk * HPC * DH:(kk + 1) * HPC * DH] for kk in range(KT)]

        # exp table warm-up
        warm = cst.tile([1, 4], F32, tag="warm")
        nc.vector.memset(warm[:], 0.0)
        nc.scalar.activation(warm[:], warm[:], AF.Exp)

        for rep in range(repeat):
            _body(nc, tc, ctx, rep, x8, wqk8, xT16, wv, aqrep, akT, s1c,
                  w2b, w2d, b2r, w3f, tri, iden, woutT, outT_d)

    nc.compile()
    return nc


def _body(nc, tc, ctx, rep, x8, wqk8, xT16, wv, aqrep, akT, s1c, w2b, w2d,
          b2r, w3f, tri, iden, woutT, outT_d):
    r = f"r{rep}"
    cst2 = ctx.enter_context(tc.tile_pool(name=f"cst2_{r}", bufs=1))

    # ------ P1: q/k projections (fp8 DoubleRow) -> qk16 [128, N] bf16 ------
    qk16 = []  # [q(2 heads), k(2 heads)]
    with tc.tile_pool(name=f"qkps_{r}", bufs=2, space="PSUM") as qkps:
        for m in range(2):
            ps = qkps.tile([128, N], F32, tag="qk")
            for kp in range(KT // 2):
                lhs = wqk8[:, kp * 512:(kp + 1) * 512] \
                    .rearrange("p (two c) -> p two c", two=2) \
                    [:, :, m * 128:(m + 1) * 128]
                rhs = x8[:, kp * 2 * N:(kp + 1) * 2 * N] \
                    .rearrange("p (two n) -> p two n", two=2)
                nc.tensor.matmul(ps[:, :], lhs, rhs,
                                 start=(kp == 0), stop=(kp == KT // 2 - 1),
                                 perf_mode=DR)
            sb = cst2.tile([128, N], BF16, tag=f"qk16_{m}")
            nc.vector.tensor_copy(sb[:], ps[:])
            qk16.append(sb)

    # -------- P3: score MLP + softmax + attn@v, heads interleaved ----------
    out_h = []  # [64, N] bf16 normalized attention output per head
    with tc.tile_pool(name=f"s2_{r}", bufs=TUNE["s2_bufs"], space="PSUM") as s2ps, \
         tc.tile_pool(name=f"sc_{r}", bufs=1, space="PSUM") as scps, \
         tc.tile_pool(name=f"op_{r}", bufs=1, space="PSUM") as ops, \
         tc.tile_pool(name=f"wk_{r}", bufs=TUNE["h1_bufs"]) as wk, \
         tc.tile_pool(name=f"h2_{r}", bufs=TUNE["h2_bufs"]) as h2p, \
         tc.tile_pool(name=f"ex_{r}", bufs=TUNE["ex_bufs"]) as exp_pool:

        a4s, cbs, op_pss = [], [], []
        for h in range(HPC):
            # a4 = 4x-replicated a^T (+ s1const via scalar add) [128, N] bf16
            a_ps = scps.tile([128, N], F32, tag=f"sc{h}")
            nc.tensor.matmul(a_ps[:, :], aqrep[64 * h:64 * (h + 1), :],
                             qk16[0][64 * h:64 * (h + 1), :],
                             start=True, stop=True, tile_position=(64 * h, 0))
            a4 = cst2.tile([128, N], BF16, tag=f"a4_{h}")
            nc.vector.tensor_scalar(a4[:], a_ps[:], s1c[:], None, ALU.add)
            a4s.append(a4)

            # cbias[32u+p, g] = (Ak k^T)[p, 4g+u]  [128, 128] f32
            c_ps = scps.tile([128, 128], F32, tag=f"sc{h}")
            k_re = qk16[1][64 * h:64 * (h + 1), :].rearrange(
                "d (g u) -> d u g", u=4)
            for u in range(4):
                nc.tensor.matmul(c_ps[32 * u:32 * (u + 1), :],
                                 akT[64 * h:64 * (h + 1), :],
                                 k_re[:, u, :], start=True, stop=True,
                                 tile_position=(64 * h, 32 * u))
            cb = cst2.tile([128, 128], F32, tag=f"cb_{h}")
            nc.vector.tensor_copy(cb[:], c_ps[:])
            cbs.append(cb)

            # out' accumulator [65, N] psum (num rows 0..64, den row 64)
            op_ps = ops.tile([65, N], F32, tag="op")
            op_pss.append(op_ps)

        # ---- v projection -> v' [128, 130] bf16 per j-tile (emitted
        # lazily inside the scoring loop to keep the early PE stream free) --
        vp = cst2.tile([128, NT * 130], BF16, tag="vp")

        def emit_vproj(t):
            ps_v = s2ps.tile([128, HPC * DH], F32, tag="s2")
            for kk in range(KT):
                nc.tensor.matmul(ps_v[:, :],
                                 xT16[kk][:, t * 128:(t + 1) * 128],
                                 wv[kk][:, :],
                                 start=(kk == 0), stop=(kk == KT - 1))
            for h in range(HPC):
                o0 = t * 130 + h * 65
                nc.scalar.copy(vp[:, o0:o0 + DH],
                               ps_v[:, h * DH:(h + 1) * DH])
                nc.vector.memset(vp[:, o0 + DH:o0 + 65], 1.0)

        s1_n = [0]  # stage-1 round-robin counter
        pair_n = [0]
        s25_n = [0]
        deferred = []  # P4 work units for finished heads, drained during
                       # the other head's scoring to fill PE bubbles
        # greedy engine load balancing: estimated busy-ns per engine.
        # Act/Pool start idle until the qk chain completes - handicap them
        # so the balancer sees wall-clock finishing times, not raw load.
        est = {"dve": 0.0, "act": 0.0, "pool": 0.0}

        def bal_copy(dst, src, cols):
            act_c = 0.833 * cols + 185
            dve_c = 1.04 * cols + 125
            if est["dve"] + dve_c < est["act"] + act_c:
                est["dve"] += dve_c
                nc.vector.tensor_copy(dst, src)
            else:
                est["act"] += act_c
                nc.scalar.copy(dst, src)

        def emit_s1(h1, a4, i0ofs, cb, g, eng):
            if eng == "act":
                nc.scalar.activation(h1, a4[:, i0ofs:N], AF.Relu,
                                     bias=cb[:, g:g + 1], scale=1.0)
            else:
                e = nc.gpsimd if eng == "pool" else nc.vector
                e.tensor_scalar(h1, a4[:, i0ofs:N], cb[:, g:g + 1], 0.0,
                                ALU.add, ALU.max)

        def emit_s25(h2ap, psap, cols):
            s25_n[0] += 1
            act_c = 0.833 * cols + 185
            dve_c = 1.04 * cols + 125
            if est["dve"] + dve_c < est["act"] + act_c:
                est["dve"] += dve_c
                nc.vector.tensor_scalar(h2ap, psap, b2r[:], 0.0,
                                        ALU.add, ALU.max)
            else:
                est["act"] += act_c
                nc.scalar.activation(h2ap, psap, AF.Relu, bias=b2r[:],
                                     scale=1.0)

        for h in range(HPC):
            for t in range(NT):
                a4, cb, op_ps = a4s[h], cbs[h], op_pss[h]
                L = N - t * 128
                i0 = t * 128
                sc_ps = scps.tile([128, L], F32, tag=f"sc{h}")
                # causal mask init (-1e30 above diagonal); stage-3 accumulates.
                nc.tensor.matmul(sc_ps[:, :], iden[:, :], tri[:, 0:L],
                                 start=True, stop=False,
                                 skip_group_check=True)
                # nm = pairs per stage-2 psum tile (2 = DR pairing unit);
                # psum tile must stay within one 2KB bank (<=512 f32 cols).
                nm = 2 if L <= 256 else 1
                s3q = []   # deferred stage-3 emissions (1 m0-double late,
                           # so PE's s2 stream never stalls on Act's s25)
                for m0 in range(0, 16, 2):
                    if deferred and m0 % 4 == 2:
                        deferred.pop(0)()
                    ofs = 8 * m0
                    Lm = L - ofs
                    h2t = h2p.tile([128, 2 * Lm], F8, tag="h2",
                                   name=f"h2t_{h}_{t}_{m0}")

                    def emit_s2(ps2ap, m, Lm, ofs):
                        # stage-1 + stage-2 for pair m into ps2ap [128, Lm]
                        pair_n[0] += 1
                        pool_c = 2 * (1.39 * Lm + 95)
                        dve_c = 2 * (0.26 * Lm + 60)
                        use_pool = (est["pool"] + pool_c
                                    < est["dve"] + dve_c)
                        if use_pool:
                            # fp8 h1 on Pool + one DoubleRow stage-2 matmul
                            est["pool"] += pool_c
                            h1p = wk.tile([128, 2 * Lm], F8, tag="h1f",
                                          name=f"h1f_{pair_n[0]}")
                            for v in range(2):
                                g = 32 * t + 2 * m + v
                                emit_s1(h1p[:, v * Lm:(v + 1) * Lm], a4,
                                        i0 + ofs, cb, g, "pool")
                            nc.tensor.matmul(
                                ps2ap,
                                w2d[:].rearrange("p (two c) -> p two c",
                                                 two=2),
                                h1p[:].rearrange("p (two n) -> p two n",
                                                 two=2),
                                start=True, stop=True, perf_mode=DR)
                        else:
                            est["dve"] += dve_c
                            for v in range(2):
                                g = 32 * t + 2 * m + v
                                h1 = wk.tile([128, Lm], BF16, tag="h1",
                                             name=f"h1_{pair_n[0]}_{v}")
                                emit_s1(h1[:], a4, i0 + ofs, cb, g, "dve")
                                nc.tensor.matmul(
                                    ps2ap[64 * v:64 * (v + 1), :],
                                    w2b[:, :], h1[:], start=True, stop=True)

                    if nm == 1:
                        for half in range(2):       # pair m0+half
                            ps2 = s2ps.tile([128, Lm], F32, tag="s2")
                            emit_s2(ps2[:, :], m0 + half, Lm, ofs)
                            emit_s25(h2t[:, half * Lm:(half + 1) * Lm],
                                     ps2[:], Lm)
                    else:
                        ps2 = s2ps.tile([128, 2 * Lm], F32, tag="s2")
                        for dm in range(2):
                            emit_s2(ps2[:, dm * Lm:(dm + 1) * Lm],
                                    m0 + dm, Lm, ofs)
                        emit_s25(h2t[:], ps2[:], 2 * Lm)
                    # stage-3: one DoubleRow matmul for pair (m0, m0+1)
                    ab, pi = m0 // 4, (m0 // 2) % 2
                    nc.tensor.matmul(
                        sc_ps[:, ofs:L],
                        w3f[2 * ab + pi].rearrange(
                            "p (two c) -> p two c", two=2),
                        h2t[:].rearrange("p (two n) -> p two n", two=2),
                        start=False, stop=(m0 + 2 >= 16),
                        skip_group_check=True, perf_mode=DR)
                if h == 0:
                    emit_vproj(t)
                ex = exp_pool.tile([128, L], BF16, tag="ex")
                est["act"] += 0.833 * L + 185
                nc.scalar.activation(ex[:], sc_ps[:], AF.Exp, scale=1.0 / SC)
                nc.tensor.matmul(op_ps[:, i0:N],
                                 vp[:, t * 130 + h * 65: t * 130 + h * 65 + 65],
                                 ex[:], start=(t == 0), stop=(t == NT - 1),
                                 skip_group_check=True)
                if t == NT - 1:
                    # normalize this head: out = num * (1/den)
                    rsb = cst2.tile([128, N], F32, tag=f"rec_{h}")
                    nc.vector.reciprocal(rsb[64:65, :], op_ps[64:65, :])
                    ones = cst2.tile([128, DH], F32, tag=f"ones_{h}")
                    nc.vector.memset(ones[64:65, :], 1.0)
                    rb_ps = scps.tile([DH, N], F32, tag=f"sc{h}")
                    nc.tensor.matmul(rb_ps[:, :], ones[64:65, :],
                                     rsb[64:65, :], start=True, stop=True)
                    rb16 = cst2.tile([DH, N], BF16, tag=f"rb16_{h}")
                    nc.scalar.copy(rb16[:], rb_ps[:])
                    o = cst2.tile([DH, N], BF16, tag=f"out_{h}")
                    nc.vector.tensor_mul(o[:], op_ps[0:DH, :], rb16[:])
                    out_h.append(o)

                    def make_p4(h, o):
                        state = {}
                        def p4_unit(ot):
                            ps = s2ps.tile([128, N], F32, tag="s2")
                            nc.tensor.matmul(
                                ps[:, :],
                                woutT[h][:, ot * 128:(ot + 1) * 128],
                                o[:, :], start=True, stop=True)
                            if ot % 2 == 0:
                                state["ob"] = wk.tile(
                                    [128, 2 * N], BF16, tag=f"ob{h}",
                                    name=f"obt_{h}_{ot}")
                                nc.vector.tensor_copy(
                                    state["ob"][:, 0:N], ps[:])
                            else:
                                ob = state["ob"]
                                nc.scalar.copy(ob[:, N:2 * N], ps[:])
                                nc.sync.dma_start(
                                    outT_d[h].rearrange(
                                        "(c a p) n -> p c a n", p=128, c=4)
                                    [:, ot // 2],
                                    ob[:].rearrange("p (a n) -> p a n", a=2))
                        return [lambda ot=ot: p4_unit(ot)
                                for ot in range(KT)]

                    deferred.extend(make_p4(h, o))

        while deferred:
            deferred.pop(0)()


# ---------------------------------------------------------------- host side -

def prep_inputs(x, Wqkv, Wout, Wq, bq, Wk, bk, W1, b1, W2, b2, W3, b3):
    """Build the per-core input maps (all numpy)."""
    x = np.asarray(x, np.float32).reshape(N, DIM)
    Wqkv = np.asarray(Wqkv, np.float32)
    Wout = np.asarray(Wout, np.float32)
    Wq, bq = np.asarray(Wq, np.float32), np.asarray(bq, np.float32)
    Wk, bk = np.asarray(Wk, np.float32), np.asarray(bk, np.float32)
    W1, b1 = np.asarray(W1, np.float32), np.asarray(b1, np.float32)
    W2, b2 = np.asarray(W2, np.float32), np.asarray(b2, np.float32)
    W3 = np.asarray(W3, np.float32)

    bf = lambda a: np.ascontiguousarray(a).astype(ml_dtypes.bfloat16)
    f8 = lambda a: np.ascontiguousarray(a).astype(ml_dtypes.float8_e4m3)
    f32 = lambda a: np.ascontiguousarray(a, np.float32)

    xT = x.T                                        # [DIM, N]
    # x8 fp8 layout [128, KT*N]: col kk*N+n, row p -> x[n, kk*128+p]*SX
    x8 = f8(xT.reshape(KT, 128, N).transpose(1, 0, 2).reshape(128, KT * N)
            * SX)

    F = SX * SW
    W1q, W1k = W1[:, :P], W1[:, P:]
    Aq = (W1q @ Wq) * (S1 / F)                      # descale fp8, scale S1
    Ak = (W1k @ Wk) * (S1 / F)
    s1const = (W1q @ bq + W1k @ bk + b1) * S1       # [32]

    aqrep = np.zeros((128, 128), np.float32)
    for u in range(4):
        aqrep[0:DH, 32 * u:32 * (u + 1)] = Aq.T
    aqrep[DH:128] = aqrep[0:DH]
    akT = np.concatenate([Ak.T, Ak.T], axis=0)      # [128, 32]

    w2b = np.zeros((128, 64), np.float32)     # blockdiag4((S4/S1)*W2^T)
    for u in range(4):
        w2b[32 * u:32 * (u + 1), 16 * u:16 * (u + 1)] = W2.T * (S4 / S1)
    # DR stage-2 weights: [128, 2, 128] fp8; i-half v targets rows 64v..
    w2drh = np.zeros((128, 2, 128), np.float32)
    for v in range(2):
        w2drh[:, v, 64 * v:64 * (v + 1)] = w2b[:, 0:64]

    ii = np.arange(128)
    tri = np.zeros((128, N), np.float32)        # [j, i]: 0 valid, -1e30 not
    tri[:, 0:128] = np.where(ii[None, :] >= ii[:, None], 0.0, -1e30)
    iden = np.eye(128, dtype=np.float32)

    cstB = np.concatenate(
        [aqrep, akT, w2b, iden, tri], axis=1)       # [128, 864]
    cstF = np.stack([np.tile(s1const, 4), np.tile(b2 * S4, 8)],
                    axis=1)                         # [128, 2] f32

    # full-height DR scatter weights: w3f[2*ab+pi][p, i, col] nonzero at
    # col = 32*ab + 8*bb + 4*v + u for bb = 2*pi + i, p = 64v+16u+q.
    w3v = W3[0] * (SC / S4)                         # [16]
    w3full = np.zeros((8, 128, 2, 128), np.float32)
    for ab in range(4):
        for pi in range(2):
            for i in range(2):
                bb = 2 * pi + i
                for v in range(2):
                    for u in range(4):
                        col = 32 * ab + 8 * bb + 4 * v + u
                        for q in range(P2):
                            w3full[2 * ab + pi, 64 * v + 16 * u + q, i,
                                   col] = w3v[q]

    # per-head channel index in Wqkv output: o = d*48 + k*16 + h
    dch = np.arange(DH)
    in_maps = []
    for c in range(N_CORES):
        h0, h1h = HPC * c, HPC * c + 1
        rows_q = [dch * 48 + 0 * HEADS + h for h in (h0, h1h)]
        rows_k = [dch * 48 + 1 * HEADS + h for h in (h0, h1h)]
        rows_v = [dch * 48 + 2 * HEADS + h for h in (h0, h1h)]
        wqkT = np.concatenate(
            [Wqkv[r] for r in rows_q + rows_k], axis=0).T     # [DIM, 256]
        # wqk8 fp8 layout [128, KT*4DH]
        wqk8 = f8(wqkT.reshape(KT, 128, 4 * DH).transpose(1, 0, 2)
                  .reshape(128, KT * 4 * DH) * SW)
        wvT = np.concatenate([Wqkv[r] for r in rows_v], axis=0).T  # [DIM,128]
        wo2 = np.concatenate(
            [Wout[:, DH * h:DH * (h + 1)].T for h in (h0, h1h)])  # [128,DIM]
        in_maps.append({
            "x8": x8,
            "wqk8": wqk8,
            "xT": bf(xT),
            "wvT": bf(wvT),
            "cstB": bf(cstB),
            "cstF": f32(cstF),
            "w3f": f8(w3full.transpose(1, 0, 2, 3).reshape(128, 8 * 256)),
            "w2d": f8(w2drh.reshape(128, 256)),
            "wo2": bf(wo2),
        })
    return in_maps


_PROGRAM_CACHE = {}


def _get_program(repeat=1):
    if repeat not in _PROGRAM_CACHE:
        _PROGRAM_CACHE[repeat] = build_program(repeat)
    return _PROGRAM_CACHE[repeat]


def run(in_maps, repeat=1):
    nc = _get_program(repeat)
    return run_bass_kernel_spmd(nc, in_maps, list(range(N_CORES)))


def kernel(**inputs) -> np.ndarray:
    in_maps = prep_inputs(**inputs)
    res = run(in_maps)
    acc = np.zeros((DIM, N), np.float64)
    for c in range(N_CORES):
        for h in range(HPC):
            acc += res.results[c][f"outT{h}"].astype(np.float64)
    return np.ascontiguousarray(acc.T.astype(np.float32)).reshape(B, N, DIM)
